# revision 6
# baseline (speedup 1.0000x reference)
"""DiffusionNCA on 8 Trainium2 NeuronCores (v3).

Strategy (hardcoded for B=4, H=W=128, C=64, HIDDEN=256, steps=10):

- Pure data parallel over 8 shards: core = (batch b, image half). Each core
  owns 64 rows of one image plus a 12-row redundantly-computed halo band, so
  NO halo exchange is needed: a 3x3 conv propagates the fake-boundary error
  1 row/step and 12 > 10 steps, so owned rows stay exact.
- Bottom-half cores store their rows REVERSED (and receive dy-flipped tap
  weights) so every core's owned region is local rows 0..63 == row-tiles
  0..15: the SPMD program is identical on all cores, and the BN AllReduce
  launches right after tile 15 while halo tiles 16..18 still compute.
- conv0/conv1/concat/fc0 are algebraically fused into 9 "tap" matmuls:
  h = sum_taps shift_tap(state) @ Wc[tap] + bias_total. Reflect padding is
  materialized as pad rows/cols of the fp16 state buffer whose upper 64
  partitions hold the +1-row-shifted image, so the 9 taps collapse to
  3 K=128 matmuls (dy in {-1,0} pairs) + 3 K=64 matmuls (dy=+1) per
  hidden-half, all reading strided windows of the same buffer (no copies).
- BatchNorm batch stats: per-core partial sums over owned pixels + a tiny
  [128,4] AllReduce per step. The BN affine is algebraically folded away:
      dx = ((h + sh2) * mask) @ (diag(scl) @ fc1)        sh2 = shift/scl
         = (h * mask) @ fc1p + mask @ G,
  with fc1p = diag(scl) @ fc1 and G = diag(sh2) @ fc1p computed per step in
  four tiny ops, so per pixel only a plain fp16 multiply remains on DVE.
- Dropout/fire masks come from jax threefry with fixed seed(42) -> input-
  independent; precomputed on host, combined, streamed as fp16 {0,1}; the
  1/(1-p) dropout scale is folded into fc1_w.
- State lives entirely in SBUF as fp16 for the whole 10-step rollout in a
  single kernel launch (in-place residual updates; lower+upper written from
  the same PSUM tile so both copies stay bit-identical).
"""

import sys

for _p in ("/opt/trn_rl_repo", "/root/.axon_site/_ro/trn_rl_repo"):
    if _p not in sys.path:
        sys.path.append(_p)

import numpy as np

C_N = 64
HIDDEN = 256
FIRE_RATE = 0.5
DROP = 0.25
EPS = 1e-5
SLOPE = 0.01

B, H, W = 4, 128, 128
NCORES = 8
HALO = 12                  # >= steps, multiple of 4
ROWS = 64 + HALO           # 76 computed rows per core
TILES = ROWS // 4          # 19 row-tiles of 4 rows (512 pixels)
OWNED_TILES = 16           # local tiles 0..15 are the owned 64 rows
WP = W + 2                 # reflect-padded width
BROWS = ROWS + 2           # + top/bottom pad rows
NPIX_TOT = float(B * H * W)

_CACHE = {}


def _build_program(steps):
    import concourse.bacc as bacc
    import concourse.mybir as mybir
    from concourse import tile

    f32 = mybir.dt.float32
    f16 = mybir.dt.float16
    AT = mybir.ActivationFunctionType
    ALU = mybir.AluOpType

    nc = bacc.Bacc("TRN2", target_bir_lowering=False, debug=False,
                   enable_asserts=True, num_devices=NCORES)

    mirror0 = nc.dram_tensor("mirror0", [128, BROWS, WP], f16, kind="ExternalInput")
    masks = nc.dram_tensor("masks", [steps, 2, TILES, 128, 512], f16, kind="ExternalInput")
    wtap = nc.dram_tensor("wtap", [128, 1536], f16, kind="ExternalInput")
    fc1w = nc.dram_tensor("fc1w", [128, 256], f16, kind="ExternalInput")
    consts = nc.dram_tensor("consts", [128, 8], f32, kind="ExternalInput")
    out_t = nc.dram_tensor("out", [2, ROWS, W], f16, kind="ExternalOutput")

    with tile.TileContext(nc, num_cores=NCORES) as tc:
        with tc.tile_pool(name="const", bufs=1) as cpool, \
             tc.tile_pool(name="work", bufs=3) as wpool, \
             tc.tile_pool(name="mpool", bufs=16) as mpool, \
             tc.tile_pool(name="small", bufs=2) as spool, \
             tc.tile_pool(name="psum", bufs=2, space="PSUM") as ppool, \
             tc.tile_pool(name="dram", bufs=2, space="DRAM") as dpool:

            w_sb = cpool.tile([128, 1536], f16)
            fc1_sb = cpool.tile([128, 256], f16)
            fc1p = cpool.tile([128, 256], f16)
            gmat = cpool.tile([128, 256], f16)
            dum = cpool.tile([128, 1], f32)
            c_sb = cpool.tile([128, 8], f32)
            mirror = cpool.tile([128, BROWS, WP], f16)
            h_sb = cpool.tile([128, 2 * TILES * 512], f16)
            stats = cpool.tile([128, 4 * OWNED_TILES], f32)

            nc.vector.memset(fc1p[:], 0.0)
            nc.vector.memset(gmat[:], 0.0)
            nc.sync.dma_start(w_sb[:], wtap[:])
            nc.sync.dma_start(fc1_sb[:], fc1w[:])
            nc.sync.dma_start(c_sb[:], consts[:])
            nc.sync.dma_start(mirror[:], mirror0[:])

            for s in range(steps):

                def pass1_tile(t, accum):
                    for hh in range(2):
                        hp = ppool.tile([128, 512], f32, tag="hp")
                        for i in range(3):          # dx = -1, 0, 1
                            nc.tensor.matmul(      # dy=-1 (lower) + dy=0 (upper)
                                hp[:],
                                w_sb[:, (i * 2 + hh) * 128:(i * 2 + hh + 1) * 128],
                                mirror[:, 4 * t: 4 * t + 4, i: i + 128],
                                start=(i == 0), stop=False)
                        for i in range(3):
                            # dy=+1 via upper half; lower weight rows are zero
                            # so K stays 128 (full array keeps HAM at K=8/8)
                            nc.tensor.matmul(
                                hp[:],
                                w_sb[:, 768 + (i * 2 + hh) * 128: 768 + (i * 2 + hh + 1) * 128],
                                mirror[:, 4 * t + 1: 4 * t + 5, i: i + 128],
                                start=False, stop=(i == 2))
                        hsl = h_sb[:, (t * 2 + hh) * 512:(t * 2 + hh + 1) * 512]
                        if accum:
                            nc.scalar.activation(
                                hsl, hp[:], AT.Lrelu, bias=c_sb[:, hh:hh + 1],
                                scale=1.0, alpha=SLOPE,
                                accum_out=stats[:, hh * OWNED_TILES + t: hh * OWNED_TILES + t + 1])
                            sq = wpool.tile([128, 512], f16, tag="sq")
                            nc.scalar.activation(
                                sq[:], hsl, AT.Square,
                                accum_out=stats[:, (2 + hh) * OWNED_TILES + t: (2 + hh) * OWNED_TILES + t + 1])
                        else:
                            nc.scalar.activation(hsl, hp[:], AT.Lrelu,
                                                 bias=c_sb[:, hh:hh + 1],
                                                 scale=1.0, alpha=SLOPE)

                # ---- pass 1 on owned tiles, then AR, then halo tiles ----
                for t in range(OWNED_TILES):
                    pass1_tile(t, True)

                loc4 = spool.tile([128, 4], f32, tag="loc4")
                nc.vector.tensor_reduce(
                    loc4[:], stats[:].rearrange("p (g t) -> p g t", t=OWNED_TILES),
                    axis=mybir.AxisListType.X, op=ALU.add)
                cin = dpool.tile([128, 4], f32, tag="cin")
                cout = dpool.tile([128, 4], f32, tag="cout")
                nc.sync.dma_start(cin[:], loc4[:])
                nc.gpsimd.collective_compute(
                    "AllReduce", ALU.add,
                    replica_groups=[list(range(NCORES))],
                    ins=[cin.opt()], outs=[cout.opt()])
                gs = spool.tile([128, 4], f32, tag="gs")
                nc.sync.dma_start(gs[:], cout[:])

                for t in range(OWNED_TILES, TILES):
                    pass1_tile(t, False)

                # preload the Sqrt activation table while the AR drains
                nc.scalar.activation(dum[:], c_sb[:, 6:7], AT.Sqrt)

                # ---- BN coefficients (all DVE except the Sqrt) ----
                mug = spool.tile([128, 4], f32, tag="mug")
                nc.vector.tensor_scalar_mul(mug[:], gs[:], 1.0 / NPIX_TOT)
                musq = spool.tile([128, 2], f32, tag="musq")
                nc.vector.tensor_mul(musq[:], mug[:, 0:2], mug[:, 0:2])
                var = spool.tile([128, 2], f32, tag="var")
                nc.vector.tensor_sub(var[:], mug[:, 2:4], musq[:])
                std = spool.tile([128, 2], f32, tag="std")
                nc.scalar.activation(std[:], var[:], AT.Sqrt, bias=c_sb[:, 6:7])
                inv = spool.tile([128, 2], f32, tag="inv")
                nc.vector.reciprocal(inv[:], std[:])
                scl = spool.tile([128, 2], f32, tag="scl")
                nc.vector.tensor_mul(scl[:], c_sb[:, 2:4], inv[:])
                rb = spool.tile([128, 2], f32, tag="rb")
                nc.vector.reciprocal(rb[:], scl[:])
                sh2a = spool.tile([128, 2], f32, tag="sh2a")
                nc.vector.tensor_mul(sh2a[:], c_sb[:, 4:6], rb[:])
                sh2 = spool.tile([128, 2], f32, tag="sh2")
                nc.vector.tensor_sub(sh2[:], sh2a[:], mug[:, 0:2])
                nc.vector.tensor_scalar_mul(fc1p[:, 0:64], fc1_sb[:, 0:64], scl[:, 0:1])
                nc.vector.tensor_scalar_mul(fc1p[:, 128:192], fc1_sb[:, 128:192], scl[:, 1:2])
                nc.vector.tensor_scalar_mul(gmat[:, 0:64], fc1p[:, 0:64], sh2[:, 0:1])
                nc.vector.tensor_scalar_mul(gmat[:, 128:192], fc1p[:, 128:192], sh2[:, 1:2])
                # swap the ACT table back to Lrelu off the critical path
                nc.scalar.activation(dum[:], c_sb[:, 6:7], AT.Lrelu, alpha=SLOPE)

                # ---- pass 2: dx = (h*mask) @ fc1p + mask @ G ; state += dx ----
                for t in range(TILES):
                    dxp = ppool.tile([128, 512], f32, tag="dxp")
                    for hh in range(2):
                        m_t = mpool.tile([128, 512], f16, tag="m")
                        nc.sync.dma_start(m_t[:], masks[s, hh, t])
                        hf = wpool.tile([128, 512], f16, tag="hf")
                        nc.vector.tensor_mul(
                            hf[:], h_sb[:, (t * 2 + hh) * 512:(t * 2 + hh + 1) * 512], m_t[:])
                        nc.tensor.matmul(dxp[:], fc1p[:, hh * 128:(hh + 1) * 128], hf[:],
                                         start=(hh == 0), stop=False)
                        nc.tensor.matmul(dxp[:], gmat[:, hh * 128:(hh + 1) * 128], m_t[:],
                                         start=False, stop=(hh == 1))
                    # upper first (reads the pre-update lower rows), then lower
                    nc.vector.tensor_add(
                        out=mirror[C_N:128, 4 * t: 4 * t + 4, 1:129],
                        in0=dxp[0:C_N, :].rearrange("p (a b) -> p a b", b=128),
                        in1=mirror[0:C_N, 1 + 4 * t: 5 + 4 * t, 1:129])
                    nc.vector.tensor_add(
                        out=mirror[0:C_N, 1 + 4 * t: 5 + 4 * t, 1:129],
                        in0=dxp[0:C_N, :].rearrange("p (a b) -> p a b", b=128),
                        in1=mirror[0:C_N, 1 + 4 * t: 5 + 4 * t, 1:129])

                # ---- reflect pads ----
                nc.gpsimd.tensor_copy(mirror[0:C_N, 0:1, 1:129], mirror[0:C_N, 2:3, 1:129])
                nc.gpsimd.tensor_copy(mirror[0:C_N, BROWS - 1:BROWS, 1:129],
                                      mirror[0:C_N, BROWS - 3:BROWS - 2, 1:129])
                nc.gpsimd.tensor_copy(mirror[C_N:128, BROWS - 2:BROWS - 1, 1:129],
                                      mirror[C_N:128, BROWS - 4:BROWS - 3, 1:129])
                nc.gpsimd.tensor_copy(mirror[:, :, 0:1], mirror[:, :, 2:3])
                nc.gpsimd.tensor_copy(mirror[:, :, WP - 1:WP], mirror[:, :, WP - 3:WP - 2])

            nc.sync.dma_start(out_t[:], mirror[0:2, 1:BROWS - 1, 1:WP - 1])

    nc.compile()
    return nc


def _host_masks(steps):
    """Combined dropout-keep & fire masks, exactly matching the reference's
    threefry stream. Input-independent (seed 42)."""
    import jax

    cpu = jax.devices("cpu")[0]
    with jax.default_device(cpu):
        keys = jax.random.split(jax.random.key(42), steps)
        out = np.empty((steps, B, H, W, HIDDEN), np.uint8)
        for s in range(steps):
            k_drop, k_fire = jax.random.split(keys[s])
            keep = jax.random.bernoulli(k_drop, 1.0 - DROP, (B, H, W, HIDDEN))
            fire = jax.random.uniform(k_fire, (B, H, W, 1)) > FIRE_RATE
            out[s] = np.asarray(keep & fire, np.uint8)
    return out


def _prep_inputs(inputs, steps):
    x = np.asarray(inputs["x"], np.float32)
    t = np.asarray(inputs["t"], np.float32)
    p0w = np.asarray(inputs["p0_w"], np.float64)
    p0b = np.asarray(inputs["p0_b"], np.float64)
    p1w = np.asarray(inputs["p1_w"], np.float64)
    p1b = np.asarray(inputs["p1_b"], np.float64)
    fc0w = np.asarray(inputs["fc0_w"], np.float64)
    fc0b = np.asarray(inputs["fc0_b"], np.float64)
    fc1w = np.asarray(inputs["fc1_w"], np.float64)
    gamma = np.asarray(inputs["bn_gamma"], np.float32)
    beta = np.asarray(inputs["bn_beta"], np.float32)

    # fused conv+fc0 tap weights, [3(dy),3(dx),64,256]
    Wc = np.zeros((3, 3, C_N, HIDDEN), np.float32)
    for r in range(3):
        for c in range(3):
            wc = p0w[r, c] @ fc0w[C_N:2 * C_N] + p1w[r, c] @ fc0w[2 * C_N:]
            if r == 1 and c == 1:
                wc = wc + fc0w[0:C_N]
            Wc[r, c] = wc.astype(np.float32)
    bias_total = (p0b @ fc0w[C_N:2 * C_N] + p1b @ fc0w[2 * C_N:] + fc0b).astype(np.float32)

    def build_wtap(flip):
        wt = np.zeros((128, 1536), np.float16)
        for i in range(3):          # dx index
            for hh in range(2):
                wm1 = Wc[2 if flip else 0, i]     # local dy=-1
                w0 = Wc[1, i]                     # local dy=0
                wp1 = Wc[0 if flip else 2, i]     # local dy=+1
                col = (i * 2 + hh) * 128
                wt[0:C_N, col:col + 128] = wm1[:, hh * 128:(hh + 1) * 128]
                wt[C_N:128, col:col + 128] = w0[:, hh * 128:(hh + 1) * 128]
                scol = 768 + (i * 2 + hh) * 128
                wt[C_N:128, scol:scol + 128] = wp1[:, hh * 128:(hh + 1) * 128]
        return wt

    wtap_by_flip = [build_wtap(False), build_wtap(True)]

    fc1s = fc1w / (1.0 - DROP)
    fc1_host = np.zeros((128, 256), np.float16)
    for hh in range(2):
        fc1_host[:, hh * 128:hh * 128 + 64] = fc1s[hh * 128:(hh + 1) * 128, :].astype(np.float16)

    consts = np.zeros((128, 8), np.float32)
    consts[:, 0] = bias_total[0:128]
    consts[:, 1] = bias_total[128:256]
    consts[:, 2] = gamma[0:128]
    consts[:, 3] = gamma[128:256]
    consts[:, 4] = beta[0:128]
    consts[:, 5] = beta[128:256]
    consts[:, 6] = EPS
    consts[:, 7] = 1.0 / NPIX_TOT

    # seed state [B,H,W,C]
    st = np.zeros((B, H, W, C_N), np.float32)
    st[..., 1] = x[:, 0]
    lin = np.linspace(0.0, 1.0, H, dtype=np.float32)
    st[..., C_N - 2] = (lin[:, None] + lin[None, :]) * 0.5
    st[..., C_N - 1] = t[0]

    keep_eff = _host_masks(steps)

    in_maps = []
    for c in range(NCORES):
        b, half = c // 2, c % 2
        flip = half == 1

        if not flip:
            sl = st[b, 0:ROWS]                       # [76,128,64] local==global
            msl = keep_eff[:, b, 0:ROWS]
        else:
            sl = st[b, H - ROWS:H][::-1]             # local i = global 127-i
            msl = keep_eff[:, b, H - ROWS:H][:, ::-1]

        p = np.concatenate([sl[1:2], sl, sl[ROWS - 2:ROWS - 1]], axis=0)
        p = np.concatenate([p[:, 1:2], p, p[:, W - 2:W - 1]], axis=1)  # [78,130,64]
        low = p.transpose(2, 0, 1).astype(np.float16)                  # [64,78,130]
        up = np.zeros_like(low)
        up[:, 0:BROWS - 1] = low[:, 1:BROWS]
        mirror0 = np.ascontiguousarray(np.concatenate([low, up], axis=0))

        m = msl.reshape(steps, TILES, 4, W, 2, 128)
        m = np.ascontiguousarray(
            m.transpose(0, 4, 1, 5, 2, 3)).reshape(steps, 2, TILES, 128, 512).astype(np.float16)

        in_maps.append({
            "mirror0": mirror0,
            "masks": m,
            "wtap": wtap_by_flip[flip],
            "fc1w": fc1_host,
            "consts": consts,
        })
    return in_maps


def _run(inputs, trace=False, **kw):
    from concourse.bass_utils import run_bass_kernel_spmd

    steps = int(np.asarray(inputs["steps"]))
    assert HALO >= steps, f"halo {HALO} < steps {steps}"
    if steps not in _CACHE:
        _CACHE[steps] = _build_program(steps)
    nc = _CACHE[steps]
    in_maps = _prep_inputs(inputs, steps)
    res = run_bass_kernel_spmd(nc, in_maps, core_ids=list(range(NCORES)),
                               trace=trace, **kw)

    out0 = np.zeros((B, 1, H, W), np.float32)
    out1 = np.zeros((B, H, W), np.float32)
    for c in range(NCORES):
        b, half = c // 2, c % 2
        r = res.results[c]["out"][:, 0:64, :].astype(np.float32)
        if half == 1:
            r = r[:, ::-1, :]
        rows = slice(half * 64, (half + 1) * 64)
        out0[b, 0, rows, :] = r[0]
        out1[b, rows, :] = r[1]
    return (out0, out1), res


def kernel(**inputs):
    (out0, out1), _ = _run(inputs)
    return out0, out1
